# revision 28
# baseline (speedup 1.0000x reference)
"""EnhancedDNCMemory forward step on 8 Trainium2 NeuronCores.

Strategy
--------
The only heavy tensor is the temporal link matrix ``link`` [B=4, N=4096,
N=4096] (256 MiB f32). Everything else is O(N) or O(N*W) and is computed on
the host in float32.

The reference computes (per batch)::

    link_new = (1 - w_i - w_j) * link + w_i * p_j          (diag zeroed)
    fwd[r]   = link_new   @ rwp[r]
    bwd[r]   = link_new^T @ rwp[r]

Expanding link_new, both contractions decompose into matvecs against the
*raw* link matrix with the 8 stacked vectors V = [rwp^T | (w*rwp)^T] (N x 8):

    fwd[r,i] = (1-w_i)*(L@rwp_r)_i - (L@(w*rwp_r))_i + w_i*(p.rwp_r) - w_i*p_i*rwp_r_i
    bwd[r,i] = (1-w_i)*(L^T@rwp_r)_i - (L^T@(w*rwp_r))_i + p_i*(w.rwp_r) - w_i*p_i*rwp_r_i

So the device only computes Y1 = L_slab @ V and Y2 = L_slab^T @ V_slab,
streaming each element of ``link`` from HBM exactly once. Sharding: 8 cores =
4 batches x 2 row-slabs of 2048 rows. Y1 slabs concatenate; Y2 partials sum
(both on host, they are [N, 8] per core).

Device kernel (per core, slab [2048, 4096]) — fp8 + DoubleRow edition:
  - Everything streams in float8_e4m3 with an exact x4096 (power-of-2) host
    prescale; f32 PSUM accumulation. Host divides the 2^24 scale back out.
    Tolerance is 2e-2; fp8 on these well-conditioned positive sums lands
    ~1e-3.
  - Matmuls run in MatmulPerfMode.DoubleRow: lhsT [128,2,8], rhs [128,2,512]
    (two 128-row k-tiles per instruction, 0.5 PE cycles per output element).
  - The transposed orientation needed for Y1 is produced on 16-bit units:
    a pair of adjacent fp8 columns (2j, 2j+1) rides as one fp16 value, so a
    [128,128]-fp16 transpose moves a [128 rows x 256 fp8 cols] block. The
    transposed tile then holds, at partition jj, the byte-interleaved pair
    of columns j=2jj(+r) — exactly DoubleRow's rhs access pattern
    [p, (2, stride 1), (n, stride 2)].
  - Transpose blocks are routed per-variant between: PE transpose-mode
    (PSUM + DVE/ACT copy to SBUF), the DMA xbar transpose from SBUF
    (dma_start_transpose), or the DMA xbar straight from DRAM.

Toolchain notes: walrus on this stack allows at most ONE sync-wait per
instruction — _legalize_waits() drops redundant same-engine waits and hoists
the rest onto same-engine NoOps, and the Tile kernel-tail drain is split
into one Drain per outstanding semaphore.
"""

import os

import numpy as np

B = 4
N = 4096
W = 64
R = 4
NCORES = 8
SLAB = N // 2  # rows per core
EPS = 1e-6

# Transpose routing variant:
#   "pe"  — all 32 transpose blocks per core on the PE (transpose mode)
#   "sx"  — all via DMA xbar transpose, SBUF->SBUF (256 small instrs)
#   "dx4" — jj-blocks {3,7,11,15} via DMA xbar straight from DRAM, rest PE
#   "htN" — N late jj-blocks host-pre-transposed and DMA-loaded, rest PE
VARIANT = os.environ.get("DNC_VARIANT", "pi")

_HT_SETS = {
    "ht4": (7, 11, 13, 15),
    "ht6": (5, 7, 9, 11, 13, 15),
    "ht8": (3, 5, 7, 9, 11, 12, 13, 15),
}
# "pe2": pe + quarter loads alternating between the SP and ACT hwdge queues
#        (halves per-tile feed latency), copies rebalanced DVE:ACT 3:1.

_NC = None
_NC_VARIANT = None
LAST_RESULT = None


# ----------------------------------------------------------------- device ---


def _build_program(variant):
    import concourse.bass as bass
    import concourse.mybir as mybir
    from concourse.tile import TileContext

    F8 = mybir.dt.float8e4
    F16 = mybir.dt.float16
    F32 = mybir.dt.float32
    DR = mybir.MatmulPerfMode.DoubleRow

    ht_blocks = ()
    split_loads = variant in ("pe2", "q32")
    quad = variant == "q32"
    hta = variant in ("hta", "htb")
    htb = variant == "htb"
    contig = variant in ("pec", "hyb4")
    pf = variant == "pf"
    pg = variant == "pg"
    pi = variant == "pi"
    if variant in ("pe", "pe2", "q32", "hta", "htb", "pec", "hyb4", "pf", "pg", "pi"):
        dx_blocks = set()
        sbuf_xbar = False
        if variant == "hyb4":
            ht_blocks = (9, 11, 13, 15)
    elif variant == "sx":
        dx_blocks = set()
        sbuf_xbar = True
    elif variant == "dx4":
        dx_blocks = {3, 7, 11, 15}
        sbuf_xbar = False
    elif variant == "dx4l":
        dx_blocks = {9, 11, 13, 15}
        sbuf_xbar = False
    elif variant == "dx6l":
        dx_blocks = {5, 7, 9, 11, 13, 15}
        sbuf_xbar = False
    elif variant == "dx8l":
        dx_blocks = set(range(8, 16))
        sbuf_xbar = False
    elif variant in _HT_SETS:
        dx_blocks = set()
        sbuf_xbar = False
        ht_blocks = _HT_SETS[variant]
    else:
        raise ValueError(variant)

    class SplitDrainTileContext(TileContext):
        """Split the kernel-tail drain: walrus caps sync-waits per inst at 1."""

        def _drain_and_barrier(self, tick_clock, wait_clock):
            from concourse.vector_clock import ScopedClock, VectorClock

            vec = list(tick_clock.global_clock)
            nz = [i for i, t in enumerate(vec) if t > 0]
            for proc in nz:
                pv = VectorClock(
                    [t if j == proc else 0 for j, t in enumerate(vec)]
                )
                d = self.nc.sync.drain()
                wait_clock.add_sem_waits(d.ins, ScopedClock({None: pv}))
            if not nz:
                d = self.nc.sync.drain()
                wait_clock.add_sem_waits(
                    d.ins, ScopedClock({None: tick_clock.global_clock})
                )
            self.nc.all_engine_barrier()
            assert self.sems is not None
            popped = self.nc._tile_sem_poison_stack.pop()
            assert popped is self._sem_poison
            self.nc.clear_and_free_semaphores(list(self.sems.allocated().values()))
            self.nc.all_engine_barrier()

    nc = bass.Bass()
    if contig:
        # tile-major layout: lmat2[t, p, c, j] = L[128*c + p, 512*t + j], so a
        # tile load is per-partition contiguous (8 KB runs, 2 KB per quarter).
        lmat = nc.dram_tensor("lmat", [8, 128, 16, 512], F8, kind="ExternalInput")
    else:
        lmat = nc.dram_tensor("lmat", [SLAB, N], F8, kind="ExternalInput")
    # consts: [vfull' (16 blocks x 2 x 32) | vslab (8 pairs x 2 x 32)].
    # V is padded 8 -> 32 columns: walrus's dual-fp8 LD_WEIGHTS ISA check
    # rejects stationary tiles narrower than 32 output partitions.
    consts = nc.dram_tensor("consts", [128, 1536], F8, kind="ExternalInput")
    tmat = None
    if ht_blocks and variant == "hyb4":
        tmat = nc.dram_tensor(
            "tmat", [len(ht_blocks) * 128, 2048], F16, kind="ExternalInput"
        )
    elif ht_blocks:
        # host-pre-transposed jj-blocks, ttile byte layout ([jj, i] fp16 pairs)
        tmat = nc.dram_tensor(
            "tmat", [len(ht_blocks) * 128, 2048], F16, kind="ExternalInput"
        )
    elif hta or variant == "htb":
        # ALL 16 jj-blocks host-pre-transposed: the PE does matmuls only.
        # tmat loads ride the scalar hwdge queue, parallel to the raw-slab
        # loads on the sync queue (4.2 MB vs 8.4 MB per core).
        tmat = nc.dram_tensor("tmat", [16 * 128, 2048], F16, kind="ExternalInput")
    y1t = nc.dram_tensor("y1t", [8, SLAB], F32, kind="ExternalOutput")
    y2t = nc.dram_tensor("y2t", [8, N], F32, kind="ExternalOutput")

    NJT = N // 512  # 8 column tiles
    NIC = SLAB // 128  # 16 row chunks

    with SplitDrainTileContext(nc) as tc:
        with (
            tc.tile_pool(name="cpool", bufs=1) as cpool,
            tc.tile_pool(name="lpool", bufs=4 if variant == "pi" else 3) as lpool,
            tc.tile_pool(name="ttpool", bufs=3) as ttpool,
            tc.tile_pool(name="spool", bufs=3) as spool,
            tc.tile_pool(name="y1pool", bufs=1, space="PSUM") as y1pool,
            tc.tile_pool(name="y2pool", bufs=1, space="PSUM") as y2pool,
            tc.tile_pool(name="tpool", bufs=3, space="PSUM") as tpool,
        ):
            use_pe_transpose = (not sbuf_xbar) or dx_blocks != set()
            ident = None
            if not sbuf_xbar and not hta:
                identt = cpool.tile([128, 128], F32 if quad else F16, name="identt")
                from concourse.masks import make_identity

                make_identity(nc, identt)
                ident = identt[:, :]

            ct = cpool.tile([128, 1536], F8)
            vfull = ct[:, 0:1024]  # 16 blocks x (2 x 32)
            vslab = ct[:, 1024:1536]  # 8 chunk-pairs x (2 x 32)

            py1 = [
                y1pool.tile([32, 512], F32, tag=f"py1_{q}", name=f"py1_{q}")
                for q in range(4)
            ]

            # PE observes the identity sem (Pool) once, so later transposes
            # carry fewer waits. Multi-wait instructions are otherwise
            # handled by _legalize_waits.
            if not sbuf_xbar and not hta:
                scr = tpool.tile(
                    [128, 1024] if pg else [128, 512],
                    F32 if quad else F16,
                    tag="tps",
                    name="touch",
                )
                nc.tensor.transpose(scr[:, 0:128], ident, ident)
                if pf:
                    # pstate warmup: keep the PE continuously busy while the
                    # first slab quarters stream in, so real work starts at
                    # the full 2.4 GHz clock instead of mid-pstate.
                    for wu in range(24):
                        nc.tensor.transpose(
                            scr[:, 128 * (wu % 4) : 128 * (wu % 4) + 128],
                            ident,
                            ident,
                        )

            if contig:
                lv = None
                lm16 = None
            else:
                lv = lmat[:, :].rearrange("(c p) (t j) -> p c t j", p=128, j=512)
                lm16 = lmat[:, :].bitcast(F16)  # [SLAB, 2048] DRAM view

            # DRAM-xbar blocks: issue up front; they only depend on HBM input
            # and stream on the scalar queue concurrently with the row loads.
            dx_tt = {}
            for b in sorted(dx_blocks):
                tt = ttpool.tile(
                    [128, 2048], F16, tag=f"ttdx{b}", bufs=1, name=f"ttdx{b}"
                )
                nc.scalar.dma_start_transpose(
                    tt[:, :], lm16[:, 128 * b : 128 * (b + 1)]
                )
                dx_tt[b] = tt

            ht_tt = {}
            if hta:
                for bb in (0, 1):
                    tt = ttpool.tile([128, 2048], F16, tag="tt", bufs=4)
                    nc.scalar.dma_start(tt, tmat[128 * bb : 128 * bb + 128, :])
                    ht_tt[bb] = tt
            ncopy = 0
            for jt in range(NJT):
                slab = lpool.tile([128, NIC, 512], F8, tag="slab")
                # quarter-granularity loads so compute can start early
                for q in range(4):
                    if htb and q == 3:
                        ldq = nc.scalar  # balance: 6.3 MB on each hwdge queue
                    elif split_loads and q % 2 == 1:
                        ldq = nc.scalar
                    else:
                        ldq = nc.sync
                    ldq.dma_start(
                        slab[:, 4 * q : 4 * q + 4, :],
                        lmat[jt, :, 4 * q : 4 * q + 4, :]
                        if contig
                        else lv[:, 4 * q : 4 * q + 4, jt, :],
                    )
                    if jt == 0 and q == 0:
                        # V constants ride the ring behind the very first
                        # quarter; nothing needs them until phase B.
                        nc.sync.dma_start(ct, consts[:, :])
                if hta and jt < NJT - 1:
                    for bb in (2 * jt + 2, 2 * jt + 3):
                        tt = ttpool.tile([128, 2048], F16, tag="tt", bufs=4)
                        nc.scalar.dma_start(
                            tt, tmat[128 * bb : 128 * bb + 128, :]
                        )
                        ht_tt[bb] = tt
                # pre-transposed block k rides behind tile k's loads; it is
                # consumed at tile ht_blocks[k]//2 > k, so it lands early.
                if jt < len(ht_blocks):
                    bb = ht_blocks[jt]
                    tt = ttpool.tile(
                        [128, 2048], F16, tag=f"ttht{bb}", bufs=1, name=f"ttht{bb}"
                    )
                    ldq2 = nc.scalar if variant == "hyb4" else nc.sync
                    ldq2.dma_start(tt, tmat[128 * jt : 128 * jt + 128, :])
                    ht_tt[bb] = tt

                # Phase A: produce the 16-bit-paired transposed tiles for the
                # two jj-blocks (h=0,1) of this column tile.
                interleave0 = (
                    jt == 0
                    and variant in ("pec", "hyb4")
                    and not quad
                    and not sbuf_xbar
                )

                def emit_pack(tt, h, p4):
                    tps = tpool.tile([128, 512], F16, tag="tps", name="tps")
                    for k in range(4):
                        ic = 4 * p4 + k
                        nc.tensor.transpose(
                            tps[:, 128 * k : 128 * k + 128],
                            slab[:, ic, 256 * h : 256 * h + 256].bitcast(F16),
                            ident,
                        )
                    dst = tt[:, 512 * p4 : 512 * p4 + 512]
                    nonlocal_ncopy = emit_pack.ncopy
                    nmod = 4 if split_loads else 3
                    if nonlocal_ncopy % nmod != nmod - 1:
                        nc.vector.tensor_copy(dst, tps)
                    else:
                        nc.scalar.copy(dst, tps)
                    emit_pack.ncopy = nonlocal_ncopy + 1

                emit_pack.ncopy = ncopy

                def emit_y2(c, py2):
                    lhsT = vslab[:, 64 * c : 64 * c + 64].rearrange(
                        "p (two m) -> p two m", two=2
                    )
                    nc.tensor.matmul(
                        py2,
                        lhsT,
                        slab[:, 2 * c : 2 * c + 2, :],
                        start=(c == 0),
                        stop=(c == 7),
                        perf_mode=DR,
                    )

                tts = {}
                if interleave0:
                    pe_hs = [h for h in (0, 1) if (2 * jt + h) not in dx_blocks]
                    for h in (0, 1):
                        b = 2 * jt + h
                        if b in dx_blocks:
                            tts[h] = dx_tt[b]
                        elif b in ht_tt:
                            tts[h] = ht_tt[b]
                        else:
                            tts[h] = ttpool.tile(
                                [128, 2048], F16, tag="tt", bufs=4, name="tt"
                            )
                    py2 = y2pool.tile([32, 512], F32, tag="py2", name="py2")
                    for q in range(4):
                        for h in pe_hs:
                            emit_pack(tts[h], h, q)
                        emit_y2(2 * q, py2)
                        emit_y2(2 * q + 1, py2)
                    ncopy = emit_pack.ncopy
                elif quad:
                    # one jj-block of 128 fp32 quad-units covers the whole
                    # 512-col tile: 16 transposes instead of 32.
                    tt = ttpool.tile([128, 2048], F32, tag="tt", bufs=3)
                    for p4 in range(4):
                        tps = tpool.tile([128, 512], F32, tag="tps", name="tps")
                        for k in range(4):
                            ic = 4 * p4 + k
                            nc.tensor.transpose(
                                tps[:, 128 * k : 128 * k + 128],
                                slab[:, ic, :].bitcast(F32),
                                ident,
                            )
                        dst = tt[:, 512 * p4 : 512 * p4 + 512]
                        if ncopy % 4 != 3:
                            nc.vector.tensor_copy(dst, tps)
                        else:
                            nc.scalar.copy(dst, tps)
                        ncopy += 1
                    tts[0] = tts[1] = tt
                else:
                  for h in (0, 1):
                    b = 2 * jt + h
                    if b in dx_blocks:
                        tts[h] = dx_tt[b]
                        continue
                    if b in ht_tt:
                        tts[h] = ht_tt[b]
                        continue
                    tt = ttpool.tile(
                        [128, 2048], F16, tag="tt", bufs=5 if pi else 4
                    )
                    if sbuf_xbar:
                        for ic in range(NIC):
                            nc.scalar.dma_start_transpose(
                                tt[:, 128 * ic : 128 * ic + 128],
                                slab[
                                    :, ic, 256 * h : 256 * h + 256
                                ].bitcast(F16),
                            )
                    elif pg:
                        for dp in range(2):
                            tpsw = tpool.tile(
                                [128, 1024], F16, tag="tps", name="tpsw"
                            )
                            for kk in range(8):
                                ic = 8 * dp + kk
                                nc.tensor.transpose(
                                    tpsw[:, 128 * kk : 128 * kk + 128],
                                    slab[
                                        :, ic, 256 * h : 256 * h + 256
                                    ].bitcast(F16),
                                    ident,
                                )
                            dst = tt[
                                :, 1024 * dp : 1024 * dp + 1024
                            ].bitcast(F32)
                            srcv = tpsw[:, :].bitcast(F32)
                            if ncopy % 3 != 2:
                                nc.vector.tensor_copy(dst, srcv)
                            else:
                                nc.scalar.copy(dst, srcv)
                            ncopy += 1
                    else:
                        for p4 in range(4):
                            tps = tpool.tile(
                                [128, 512], F16, tag="tps", name="tps"
                            )
                            for k in range(4):
                                ic = 4 * p4 + k
                                nc.tensor.transpose(
                                    tps[:, 128 * k : 128 * k + 128],
                                    slab[
                                        :, ic, 256 * h : 256 * h + 256
                                    ].bitcast(F16),
                                    ident,
                                )
                            dst = tt[:, 512 * p4 : 512 * p4 + 512]
                            nmod = 4 if split_loads else 3
                            if pf or ncopy % nmod != nmod - 1:
                                nc.vector.tensor_copy(dst, tps)
                            else:
                                nc.scalar.copy(dst, tps)
                            ncopy += 1
                    tts[h] = tt

                # Phase B: one uninterrupted matmul-mode run. For hta, Y1
                # first (its tiles landed a tile ago, no slab dependency);
                # otherwise Y2 first (raw slab rhs) to give the transpose
                # copies slack.
                def emit_y2_tile():
                    py2 = y2pool.tile([32, 512], F32, tag="py2", name="py2")
                    for c in range(8):
                        emit_y2(c, py2)
                    y2s = spool.tile(
                        [8, 512], F32, tag="y2s", bufs=NJT, name="y2s"
                    )
                    nc.vector.tensor_copy(y2s, py2[0:8, :])
                    if variant in ("pe", "pf"):
                        nc.scalar.dma_start(
                            y2t[:, jt * 512 : (jt + 1) * 512], y2s
                        )
                    else:
                        nc.sync.dma_start(y2t[:, jt * 512 : (jt + 1) * 512], y2s)

                if interleave0:
                    y2s = spool.tile(
                        [8, 512], F32, tag="y2s", bufs=NJT, name="y2s"
                    )
                    nc.vector.tensor_copy(y2s, py2[0:8, :])
                    if variant == "pe":
                        nc.scalar.dma_start(
                            y2t[:, jt * 512 : (jt + 1) * 512], y2s
                        )
                    else:
                        nc.sync.dma_start(y2t[:, jt * 512 : (jt + 1) * 512], y2s)
                elif not hta:
                    emit_y2_tile()

                for h in (0, 1):
                    b = 2 * jt + h
                    lhsT = vfull[:, 64 * b : 64 * b + 64].rearrange(
                        "p (two m) -> p two m", two=2
                    )
                    for q in range(4):
                        if quad:
                            rhs = (
                                tts[h][:, 512 * q : 512 * q + 512]
                                .bitcast(F8)
                                .rearrange("p (n c) -> p c n", c=4)
                            )[:, 2 * h : 2 * h + 2, :]
                        else:
                            rhs = (
                                tts[h][:, :]
                                .bitcast(F8)[:, 1024 * q : 1024 * q + 1024]
                                .rearrange("p (n two) -> p two n", two=2)
                            )
                        nc.tensor.matmul(
                            py1[q],
                            lhsT,
                            rhs,
                            start=(b == 0),
                            stop=(b == 15),
                            perf_mode=DR,
                        )
                    if b == 15:
                        for q in range(4):
                            y1s = spool.tile(
                                [8, 512], F32, tag="y1s", bufs=4, name="y1s"
                            )
                            if variant in ("pe", "pf") or q % 2 == 0:
                                nc.vector.tensor_copy(y1s, py1[q][0:8, :])
                            else:
                                nc.scalar.copy(y1s, py1[q][0:8, :])
                            if variant in ("pe", "pf"):
                                nc.scalar.dma_start(
                                    y1t[:, q * 512 : (q + 1) * 512], y1s
                                )
                            else:
                                nc.sync.dma_start(
                                    y1t[:, q * 512 : (q + 1) * 512], y1s
                                )
                if hta:
                    emit_y2_tile()
    return nc


def _legalize_waits(nc):
    """Walrus on this toolchain allows at most ONE sync-wait per instruction.

    Two rewrites, applied to the finished BIR:
      1. Drop same-engine waits — every engine queue executes (and completes
         compute instructions) in order, so a wait on the engine's own
         semaphore from within its own stream is implied by program order.
      2. If an instruction still carries more than one wait, hoist all but
         the last onto fresh same-engine InstNoOps inserted just before it.
    """
    import concourse.mybir as mybir

    eng_prefix = {
        mybir.EngineType.PE: "PE_",
        mybir.EngineType.DVE: "DVE_",
        mybir.EngineType.Activation: "ACT_",
        mybir.EngineType.Pool: "Pool_",
        mybir.EngineType.SP: "SP_",
    }
    uid = 0
    for f in nc.m.functions:
        for b in f.blocks:
            out = []
            for inst in b.instructions:
                si = getattr(inst, "sync_info", None)
                waits = list(si.on_wait) if si is not None and si.on_wait else []
                if len(waits) > 1:
                    pref = eng_prefix.get(inst.engine)
                    if pref is not None:
                        keep = [
                            w
                            for w in waits
                            if not (w.ant_name or "").startswith(pref)
                        ]
                        waits = keep if keep else waits[-1:]
                    for w in waits[:-1]:
                        uid += 1
                        out.append(
                            mybir.InstNoOp(
                                name=f"lw-nop-{uid}",
                                engine=inst.engine,
                                sync_info=mybir.SyncInfo(
                                    on_wait=[w], on_update=[]
                                ),
                                bass_nofuse=True,
                            )
                        )
                    inst.sync_info = mybir.SyncInfo(
                        on_wait=waits[-1:],
                        on_update=list(si.on_update or []),
                    )
                out.append(inst)
            b.instructions[:] = out


def _get_nc():
    global _NC, _NC_VARIANT
    if _NC is None or _NC_VARIANT != VARIANT:
        nc = _build_program(VARIANT)
        _legalize_waits(nc)
        _NC = nc
        _NC_VARIANT = VARIANT
    return _NC


def _execute(nc, in_maps, trace):
    from concourse.bass_utils import run_bass_kernel_spmd

    return run_bass_kernel_spmd(
        nc,
        in_maps,
        list(range(NCORES)),
        trace=trace,
    )


# ------------------------------------------------------------- host math ---


def _sigmoid(x):
    x = np.asarray(x, np.float32)
    out = np.empty_like(x)
    pos = x >= 0
    out[pos] = 1.0 / (1.0 + np.exp(-x[pos]))
    ex = np.exp(x[~pos])
    out[~pos] = ex / (1.0 + ex)
    return out


def _softplus(x):
    x = np.asarray(x, np.float32)
    return np.log1p(np.exp(-np.abs(x))) + np.maximum(x, 0.0)


def _softmax(x, axis=-1):
    x = np.asarray(x, np.float32)
    m = np.max(x, axis=axis, keepdims=True)
    e = np.exp(x - m)
    return e / np.sum(e, axis=axis, keepdims=True)


def _content_weights(mem, keys, beta):
    # mem: [B,N,W], keys: [B,K,W], beta: [B,K] -> [B,K,N]
    dot = np.einsum("bnw,bkw->bkn", mem, keys, dtype=np.float32)
    mem_n = np.linalg.norm(mem, axis=-1)[:, None, :].astype(np.float32)
    key_n = np.linalg.norm(keys, axis=-1)[:, :, None].astype(np.float32)
    sim = dot / (mem_n * key_n + EPS)
    return _softmax(beta[..., None] * sim, axis=-1)


def _allocation(usage):
    idx = np.argsort(usage, axis=-1, kind="stable")
    sorted_u = np.take_along_axis(usage, idx, axis=-1)
    cp = np.cumprod(sorted_u, axis=-1)
    excl = np.concatenate([np.ones_like(cp[:, :1]), cp[:, :-1]], axis=-1)
    alloc_sorted = ((1.0 - sorted_u) * excl).astype(np.float32)
    out = np.empty_like(alloc_sorted)
    np.put_along_axis(out, idx, alloc_sorted, axis=-1)
    return out


def _make_tmat(link8_slab, ht_blocks):
    """Pre-transposed jj-blocks in ttile byte layout: tm[k*128+jj, i] is the
    fp16 pair (L[i, 256*b+2*jj], L[i, 256*b+2*jj+1]) for b = ht_blocks[k]."""
    u8 = link8_slab.view(np.uint8)  # [SLAB, N]
    parts = []
    for bb in ht_blocks:
        seg = u8[:, 256 * bb : 256 * bb + 256]  # [SLAB, 256]
        parts.append(
            np.ascontiguousarray(
                seg.reshape(SLAB, 128, 2).transpose(1, 0, 2).reshape(128, 2 * SLAB)
            )
        )
    return np.concatenate(parts, axis=0).view(np.float16)


def _make_consts(v_full, v_slab, quad=False):
    """Build the [128, 1536] fp8 consts tile from padded V ([N,32]/[SLAB,32]).

    vfull:  consts[p, 64*b + 32*r + m] = v_full[256*b + 2*p + r, m]
        (quad: consts[p, 64*(2*t+off) + 32*r + m] = v_full[512*t+4*p+2*off+r, m])
    vslab:  consts[p, 1024 + 64*c + 32*r + m] = v_slab[128*(2*c+r) + p, m]
    """
    if quad:
        vf = np.ascontiguousarray(
            v_full.reshape(8, 128, 2, 2, 32)
            .transpose(1, 0, 2, 3, 4)
            .reshape(128, 1024)
        )
    else:
        vf = np.ascontiguousarray(
            v_full.reshape(16, 128, 2, 32).transpose(1, 0, 2, 3).reshape(128, 1024)
        )
    vs = np.ascontiguousarray(
        v_slab.reshape(8, 2, 128, 32).transpose(2, 0, 1, 3).reshape(128, 512)
    )
    return np.concatenate([vf, vs], axis=1)


# ----------------------------------------------------------------- kernel ---


def kernel(
    memory,
    usage,
    link,
    precedence,
    read_w_prev,
    write_w_prev,
    write_key,
    write_strength_raw,
    erase_raw,
    write_vec,
    free_raw,
    alloc_gate_raw,
    write_gate_raw,
    read_keys,
    read_strengths_raw,
    read_modes_raw,
):
    global LAST_RESULT
    import ml_dtypes

    f32 = np.float32
    memory = np.asarray(memory, f32)
    usage = np.asarray(usage, f32)
    link = np.asarray(link, f32)
    precedence = np.asarray(precedence, f32)
    read_w_prev = np.asarray(read_w_prev, f32)
    write_w_prev = np.asarray(write_w_prev, f32)
    write_key = np.asarray(write_key, f32)
    write_strength_raw = np.asarray(write_strength_raw, f32)
    erase_raw = np.asarray(erase_raw, f32)
    write_vec = np.asarray(write_vec, f32)
    free_raw = np.asarray(free_raw, f32)
    alloc_gate_raw = np.asarray(alloc_gate_raw, f32)
    write_gate_raw = np.asarray(write_gate_raw, f32)
    read_keys = np.asarray(read_keys, f32)
    read_strengths_raw = np.asarray(read_strengths_raw, f32)
    read_modes_raw = np.asarray(read_modes_raw, f32)

    # --- interface activations ---
    write_strength = 1.0 + _softplus(write_strength_raw)  # [B]
    read_strengths = 1.0 + _softplus(read_strengths_raw)  # [B,R]
    erase = _sigmoid(erase_raw)  # [B,W]
    free = _sigmoid(free_raw)  # [B,R]
    g_a = _sigmoid(alloc_gate_raw)[:, None]  # [B,1]
    g_w = _sigmoid(write_gate_raw)[:, None]  # [B,1]
    modes = _softmax(read_modes_raw, axis=-1)  # [B,R,3]

    # --- write content addressing ---
    c_w = _content_weights(memory, write_key[:, None, :], write_strength[:, None])[
        :, 0
    ]  # [B,N]

    # --- usage update + allocation ---
    retention = np.prod(
        1.0 - free[..., None] * read_w_prev, axis=1, dtype=f32
    )  # [B,N]
    usage_new = ((usage + write_w_prev - usage * write_w_prev) * retention).astype(f32)
    alloc = _allocation(usage_new)  # [B,N]

    # --- write weights, memory erase/write ---
    w_w = (g_w * (g_a * alloc + (1.0 - g_a) * c_w)).astype(f32)  # [B,N]
    memory_new = (
        memory * (1.0 - w_w[:, :, None] * erase[:, None, :])
        + w_w[:, :, None] * write_vec[:, None, :]
    ).astype(f32)  # [B,N,W]

    # --- device part: Y1 = L @ V, Y2 = L^T @ V (per batch, split in 2 slabs) ---
    # V = [rwp^T | (w*rwp)^T]  ->  [N, 8]
    V = np.concatenate(
        [
            read_w_prev.transpose(0, 2, 1),  # [B,N,R]
            (w_w[:, :, None] * read_w_prev.transpose(0, 2, 1)),
        ],
        axis=2,
    ).astype(f32)  # [B,N,8]

    # Device runs fp8_e4m3 with an exact power-of-2 prescale: values of link
    # and V are O(1/N), so x4096 recenters them into e4m3's well-conditioned
    # range. The output scale (4096^2 = 2^24) divides out exactly.
    SCALE = 4096.0
    f8 = ml_dtypes.float8_e4m3
    Vp = np.zeros((B, N, 32), np.float32)
    Vp[:, :, :8] = V * SCALE
    V8 = Vp.astype(f8)
    link8 = (link * SCALE).astype(f8)
    in_maps = []
    for core in range(NCORES):
        b, h = divmod(core, 2)
        r0 = h * SLAB
        consts = _make_consts(V8[b], V8[b, r0 : r0 + SLAB], quad=(VARIANT == "q32"))
        in_maps.append(
            {
                "lmat": np.ascontiguousarray(link8[b, r0 : r0 + SLAB, :]),
                "consts": consts,
            }
        )

    ht_blocks = _HT_SETS.get(VARIANT, ())
    if VARIANT in ("hta", "htb"):
        ht_blocks = tuple(range(16))
    elif VARIANT == "hyb4":
        ht_blocks = (9, 11, 13, 15)
    if ht_blocks:
        for core in range(NCORES):
            b, h = divmod(core, 2)
            r0 = h * SLAB
            in_maps[core]["tmat"] = _make_tmat(link8[b, r0 : r0 + SLAB], ht_blocks)
    if VARIANT in ("pec", "hyb4"):
        for core in range(NCORES):
            lm = in_maps[core]["lmat"]  # [SLAB, N]
            in_maps[core]["lmat"] = np.ascontiguousarray(
                lm.reshape(16, 128, 8, 512).transpose(2, 1, 0, 3)
            )

    nc = _get_nc()
    res = _execute(nc, in_maps, trace=bool(os.environ.get("DNC_TRACE")))
    LAST_RESULT = res

    UNSCALE = np.float32(1.0 / (SCALE * SCALE))
    Y1 = np.empty((B, N, 8), f32)
    Y2 = np.zeros((B, N, 8), f32)
    for core in range(NCORES):
        b, h = divmod(core, 2)
        r0 = h * SLAB
        Y1[b, r0 : r0 + SLAB] = res.results[core]["y1t"].T * UNSCALE
        Y2[b] += res.results[core]["y2t"].T * UNSCALE

    A = Y1[..., :R].transpose(0, 2, 1)  # [B,R,N] = (L @ rwp_r)_i
    Bm = Y1[..., R:].transpose(0, 2, 1)  # (L @ (w*rwp_r))_i
    C = Y2[..., :R].transpose(0, 2, 1)  # (L^T @ rwp_r)_i
    D = Y2[..., R:].transpose(0, 2, 1)  # (L^T @ (w*rwp_r))_i

    w = w_w[:, None, :]  # [B,1,N]
    p = precedence[:, None, :]  # [B,1,N]
    s = np.einsum("bn,brn->br", precedence, read_w_prev, dtype=f32)[..., None]
    t = np.einsum("bn,brn->br", w_w, read_w_prev, dtype=f32)[..., None]
    diag = (w * p * read_w_prev).astype(f32)  # [B,R,N]

    fwd_w = ((1.0 - w) * A - Bm + w * s - diag).astype(f32)
    bwd_w = ((1.0 - w) * C - D + p * t - diag).astype(f32)

    # --- read content addressing + combine ---
    c_r = _content_weights(memory_new, read_keys, read_strengths)  # [B,R,N]
    read_w = (
        modes[..., 0:1] * bwd_w + modes[..., 1:2] * c_r + modes[..., 2:3] * fwd_w
    ).astype(f32)
    read_vectors = np.einsum("brn,bnw->brw", read_w, memory_new, dtype=f32)
    return read_vectors.astype(f32)
